# revision 17
# baseline (speedup 1.0000x reference)
"""Trainium2 Bass kernel for BaseTensorMemory (delta-rule tensor memory).

Computes, for full inputs queries/keys/values [B,S,D], M [D,D], z [D]:
  sigma_k = elu(keys)+1 ; existing = (sigma_k@M)/(sigma_k@z+eps)
  delta_m = clip(einsum('bsd,bse->de', sigma_k, values-existing)/(B*S), +-1)
  delta_z = sigma_k.sum((0,1))/B
  M' = clip(M+delta_m, +-100); z' = clip(z+delta_z, eps, 1e6)
  out = (sigma_q@M')/(sigma_q@z'+eps)

Strategy: data-parallel over 8 NeuronCores. Flatten B*S tokens, shard
contiguously. Per core: keys pass accumulates [sigma^T@v | sigma^T@[1|ex]]
into PSUM, AllReduce the tiny [64,129] partial, build M'/z' on-chip, then
the queries retrieve pass streams the output.

Engine budget (measured): ACT = 1 elem/part/cycle @1.2GHz dtype-blind;
DVE = 1 elem/cycle @0.96GHz, 2x when every src/dst is 2-byte unit-stride
(tensor_scalar / tensor_tensor / copy only — scalar_tensor_tensor never).
So: q/k/v are DMA-cast f32->f16 on load (SWDGE), elu+1 is one ACT exp +
DVE ts(max,+1) + DVE tt(min), both 2x-eligible.  eps dropped (norms are
O(10)+); the ex sign is folded into a subtract at update time so the
reciprocal reads the PSUM norm column directly.

Device quirk found empirically: matmuls whose operands alternate base
partition (0 vs 64) inside one PSUM accumulation group hard-crash the
device (NRT_EXEC_UNIT_UNRECOVERABLE). Retrieval is therefore parity-banked:
even token-groups accumulate in bank 0 (operands at base partition 0), odd
groups in bank 1 (base partition 64).
"""

import numpy as np

B, S, D = 16, 16384, 64
N_CORES = 8
EPS = 1e-6
MAX_DELTA = 1.0
MAX_MEMORY = 100.0
MAX_NORM = 1e6

TILE_TOKENS = 4096  # macro-tile: [128, 2048] f16, four 1024-token units
QPM = TILE_TOKENS // 128  # 32 token-groups per macro-tile
UPT = 4  # units (psum-sized, 1024 tokens) per macro-tile
VW = 2 * D + 2  # 130: [v | 1 | ex | pad] block per group (pad: 4B align)


def _build(n_cores, tokens_per_core):
    import concourse.bacc as bacc
    import concourse.mybir as mybir
    import concourse.tile as tile
    from concourse import masks

    dt = mybir.dt
    f32, f16 = dt.float32, dt.float16
    A = mybir.AluOpType
    F = mybir.ActivationFunctionType

    T = tokens_per_core
    NT = T // TILE_TOKENS
    assert NT * TILE_TOKENS == T
    MFD = QPM * D  # 2048: macro-tile free dim

    nc = bacc.Bacc(
        "TRN2", target_bir_lowering=False, debug=False, num_devices=n_cores
    )
    k_d = nc.dram_tensor("keys", [T, D], f32, kind="ExternalInput").ap()
    v_d = nc.dram_tensor("values", [T, D], f32, kind="ExternalInput").ap()
    q_d = nc.dram_tensor("queries", [T, D], f32, kind="ExternalInput").ap()
    m_d = nc.dram_tensor("m", [D, D], f32, kind="ExternalInput").ap()
    z_d = nc.dram_tensor("z", [D, 1], f32, kind="ExternalInput").ap()
    o_d = nc.dram_tensor("out", [T, D], dt.bfloat16, kind="ExternalOutput").ap()

    kr = k_d.rearrange("(n p q) d -> n p (q d)", p=128, q=QPM)
    vr = v_d.rearrange("(n p q) d -> n p (q d)", p=128, q=QPM)
    qr = q_d.rearrange("(n p q) d -> n p (q d)", p=128, q=QPM)
    orr = o_d.rearrange("(n p q) d -> n p (q d)", p=128, q=QPM)

    with tile.TileContext(nc) as tc:
        with (
            tc.tile_pool(name="const", bufs=1) as cpool,
            tc.tile_pool(name="io", bufs=3) as io,
            tc.tile_pool(name="work", bufs=4) as work,
            tc.tile_pool(name="small", bufs=6) as small,
            tc.tile_pool(name="sigq", bufs=20) as sigq,
            tc.tile_pool(name="psT", bufs=3, space="PSUM") as psTp,
            tc.tile_pool(name="psR", bufs=2, space="PSUM") as psRp,
            tc.tile_pool(name="psA", bufs=1, space="PSUM") as psAp,
            tc.tile_pool(name="dram", bufs=1, space="DRAM") as dram,
        ):
            ident = cpool.tile([128, 128], f16)
            masks.make_identity(nc, ident[:])

            # [M|z] in fp16, replicated on both partition halves (parity-
            # banked retrieve uses operands at base partition 0 and 64).
            mz = cpool.tile([128, 65], f32)
            nc.sync.dma_start(mz[0:64, 0:64], m_d[:])
            nc.sync.dma_start(mz[0:64, 64:65], z_d[:])
            nc.sync.dma_start(mz[64:128, 0:64], m_d[:])
            nc.sync.dma_start(mz[64:128, 64:65], z_d[:])
            mz16 = cpool.tile([128, 65], f16)
            nc.scalar.copy(mz16[:], mz[:])

            AW = 2 * D + 1  # psA: [sv(64) | sum_sig | ex(64)]
            psA = psAp.tile([64, AW], f32)

            ncopy = [0]  # alternate sigT copies between DVE and ACT

            def elu_tile(xt):
                """sigma = min(exp(x), max(x,0)+1) -> [128, MFD] f16.
                One ACT pass + two 2x-eligible DVE passes (all-f16)."""
                e16 = work.tile([128, MFD], f16, tag="e")
                nc.scalar.activation(e16[:], xt[:], F.Exp)
                r1 = work.tile([128, MFD], f16, tag="r")
                nc.vector.tensor_scalar(
                    r1[:], xt[:], 0.0, 1.0, op0=A.max, op1=A.add
                )
                sig = work.tile([128, MFD], f16, tag="sig")
                nc.vector.tensor_tensor(sig[:], r1[:], e16[:], op=A.min)
                return sig

            def transpose_half(sig, a, copy_eng="alt", tag="sigT"):
                """4x [128,128] PE transposes of unit a -> sigT [128, 512]
                f16 (token-groups parity-stacked on partitions). start=True
                lazily zeroes the whole PSUM bank: first matmul only."""
                psT = psTp.tile([128, 512], f16, tag="psT")
                for c in range(4):
                    nc.tensor.matmul(
                        psT[:, c * 128 : (c + 1) * 128],
                        sig[:, a * 512 + c * 128 : a * 512 + (c + 1) * 128],
                        ident[:],
                        is_transpose=True,
                        start=(c == 0),
                        stop=(c == 3),
                    )
                pool = sigq if tag == "sigTq" else work
                sigT = pool.tile([128, 512], f16, tag=tag)
                if copy_eng == "alt":
                    eng = "act" if ncopy[0] % 2 else "dve"
                    ncopy[0] += 1
                else:
                    eng = copy_eng
                if eng == "act":
                    nc.scalar.copy(sigT[:], psT[:])
                else:
                    nc.vector.tensor_copy(sigT[:], psT[:])
                return sigT

            def retrieve_half(sigT, mztile):
                """Parity-banked: even groups -> bank0 (base 0), odd ->
                bank1 (base 64). In-unit group (h par, j) = 2j+h at bank h
                col 65j. Returns psum + data/norm views [128, h, j, *]."""
                psR = psRp.tile([128, 1024], f32, tag="psR")
                for g in range(8):
                    par, j = g % 2, g // 2
                    off = par * 512 + 65 * j
                    base = par * 64
                    lhsT = sigT[base : base + 64, j * 128 : (j + 1) * 128]
                    nc.tensor.matmul(
                        psR[:, off : off + 65],
                        lhsT,
                        mztile[base : base + 64, :],
                        start=(j == 0),
                        stop=(j == 3),
                    )
                ret = psR[:].rearrange("p (h x) -> p h x", h=2)[:, :, 0:260]
                ret = ret.rearrange("p h (j c) -> p h j c", j=4)
                return psR, ret[:, :, :, 0:64], ret[:, :, :, 64:65]

            def recip_norm(normv):
                """1/norm broadcast view [128, h, j, 64] f32 (eps dropped:
                norms are O(10)+ for keys, O(1e6) for queries)."""
                rn = small.tile([128, 8], f32, tag="rn")
                rnv = rn[:].rearrange("p (h j) -> p h j", h=2).unsqueeze(3)
                nc.vector.reciprocal(rnv, normv)
                return rnv.broadcast_to((128, 2, 4, 64))

            # ---------------- keys phase ----------------
            # Stage pipeline over 1024-token units:
            #   front(h): transposes + sigT copy
            #   mid(h):   retrieve + recip + ex-mul   (1 unit behind front)
            #   back(h):  16 delta matmuls            (2 units behind)
            NH = UPT * NT
            fronts = {}
            first_mm = [True]

            def front_k(h):
                i, a = h // UPT, h % UPT
                if a == 0:
                    kt = io.tile([128, MFD], f16, tag="kt")
                    nc.gpsimd.dma_start(kt[:], kr[i])  # f32->f16 cast DMA
                    vt = io.tile([128, MFD], f16, tag="vt")
                    nc.gpsimd.dma_start(vt[:], vr[i])  # f32->f16 cast DMA
                    sig = elu_tile(kt)
                    vox = io.tile([128, QPM * VW], f16, tag="vox")
                    voxg = vox[:].rearrange("p (g c) -> p g c", g=QPM)
                    # one delta matmul per group needs [v|1|ex] contiguous;
                    # values land via ACT copy (ACT has slack), ones memset.
                    nc.scalar.copy(voxg[:, :, 0:64], vt[:])
                    nc.gpsimd.memset(voxg[:, :, 64:65], 1.0)
                    fronts[i] = (sig, vox)
                sig, vox = fronts[i]
                sigT = transpose_half(sig, a)
                return sigT

            def mid_k(h, sigT):
                i, a = h // UPT, h % UPT
                sig, vox = fronts[i]
                psR, data, normv = retrieve_half(sigT, mz16)
                rn = recip_norm(normv)
                exv = vox[:, a * 8 * VW : (a + 1) * 8 * VW].rearrange(
                    "p (j h c) -> p h j c", j=4, h=2
                )[:, :, :, 65:129]
                nc.vector.tensor_tensor(exv, data, rn, op=A.mult)

            def back_k(h, last):
                i, a = h // UPT, h % UPT
                sig, vox = fronts[i]
                for g in range(8):
                    q = a * 8 + g
                    nc.tensor.matmul(
                        psA[:, 0 : 2 * D + 1],
                        sig[:, q * 64 : (q + 1) * 64],
                        vox[:, q * VW : q * VW + 2 * D + 1],
                        start=first_mm[0],
                        stop=(last and g == 7),
                    )
                    first_mm[0] = False

            # Deep skew: mid lags front by 2 units, back lags by 4, so each
            # cross-engine semaphore hop has slack and no engine stalls on
            # the just-produced tile.
            MSK, BSK = 1, 2
            stages = []
            for h in range(NH):
                stages.append(("f", h))
                if h >= MSK:
                    stages.append(("m", h - MSK))
                if h >= BSK:
                    stages.append(("b", h - BSK))
            for h in range(NH - MSK, NH):
                stages.append(("m", h))
            for h in range(NH - BSK, NH):
                stages.append(("b", h))
            sigTs = {}
            for kind, h in stages:
                if kind == "f":
                    sigTs[h] = front_k(h)
                elif kind == "m":
                    mid_k(h, sigTs[h])
                else:
                    back_k(h, last=(h == NH - 1))

            # ---------------- allreduce + update ----------------
            # Staging DMAs ride the sync (HWDGE) queue so the gpsimd queue
            # holds only collective_compute itself; the SKEW query fronts'
            # cast-DMAs are emitted BEFORE it (see below) so query loads
            # are not serialized behind the 16us collective.
            accsb = cpool.tile([64, AW], f32)
            nc.vector.tensor_copy(accsb[:], psA[:])
            arsb = accsb
            arin = arout = None
            if n_cores > 1:
                arin = dram.tile([64, AW], f32)
                arout = dram.tile([n_cores * 64, AW], f32)
                nc.sync.dma_start(arin[:], accsb[:])

            def emit_allreduce():
                if n_cores == 1:
                    return accsb
                nc.gpsimd.collective_compute(
                    "AllGather",
                    mybir.AluOpType.bypass,
                    replica_groups=[list(range(n_cores))],
                    ins=[arin.opt()],
                    outs=[arout.opt()],
                )
                # arout rank-major [8*64, AW]; land as [64, (c w)] in SBUF
                gat = cpool.tile([64, n_cores * AW], f32)
                for c in range(n_cores):
                    nc.sync.dma_start(
                        gat[:, c * AW : (c + 1) * AW],
                        arout[c * 64 : (c + 1) * 64, :],
                    )
                ar = cpool.tile([64, AW], f32)
                gv = gat[:].rearrange("p (c w) -> p w c", c=n_cores)
                nc.vector.tensor_reduce(
                    ar[:], gv, axis=mybir.AxisListType.X,
                    op=mybir.AluOpType.add,
                )
                return ar

            def update_math():
                # Fused DVE ops; emitted after SKEW queries fronts so the
                # DVE stream has front work while the AllReduce runs.
                # arsb = [sv(64) | sum_sig(1) | ex_sum(64)]
                # delta_m = clip((sv-ex)/(B*S), +-1); M' = clip(M+dm, +-100)
                nc.vector.tensor_tensor(
                    mzn[:, 0:64], arsb[:, 0:64], arsb[:, 65:129], op=A.subtract
                )
                nc.vector.tensor_scalar(
                    mzn[:, 0:64], mzn[:, 0:64], 1.0 / (B * S), MAX_DELTA,
                    op0=A.mult, op1=A.min,
                )
                nc.vector.scalar_tensor_tensor(
                    mzn[:, 0:64], mzn[:, 0:64], -MAX_DELTA, mz[0:64, 0:64],
                    op0=A.max, op1=A.add,
                )
                nc.vector.tensor_scalar(
                    mzn[:, 0:64], mzn[:, 0:64], MAX_MEMORY, -MAX_MEMORY,
                    op0=A.min, op1=A.max,
                )
                # delta_z = acc_z/B; z' = clip(z+dz, eps, 1e6)
                nc.vector.scalar_tensor_tensor(
                    mzn[:, 64:65], arsb[:, 64:65], 1.0 / B, mz[0:64, 64:65],
                    op0=A.mult, op1=A.add,
                )
                nc.vector.tensor_scalar(
                    mzn[:, 64:65], mzn[:, 64:65], EPS, MAX_NORM,
                    op0=A.max, op1=A.min,
                )

            mzn = cpool.tile([64, 65], f32)
            mzn128 = cpool.tile([128, 65], f32)
            mzn16 = cpool.tile([128, 65], f16)

            def update_cast():
                nc.gpsimd.dma_start(mzn128[0:64, :], mzn[:])
                nc.gpsimd.dma_start(mzn128[64:128, :], mzn[:])
                nc.gpsimd.tensor_copy(mzn16[:], mzn128[:])

            # ---------------- queries phase ----------------
            outs = {}
            nq = [0]

            def front_q(h):
                i, a = h // UPT, h % UPT
                if a == 0:
                    qt = io.tile([128, MFD], f16, tag="kt")
                    nc.gpsimd.dma_start(qt[:], qr[i])  # f32->f16 cast DMA
                    sig = elu_tile(qt)
                    fronts[i + NT] = (sig, None, None)
                sig, _, _ = fronts[i + NT]
                nq[0] += 1
                return transpose_half(sig, a, tag="sigTq")

            def mid_q(h):
                i, a = h // UPT, h % UPT
                psR, data, normv = retrieve_half(sigTs[h], mzn16)
                rn = recip_norm(normv)
                if a == 0:
                    ot = io.tile([128, MFD], dt.bfloat16, tag="ot")
                    outs[i] = ot
                ot = outs[i]
                otv = ot[:, a * 512 : (a + 1) * 512].rearrange(
                    "p (j h c) -> p h j c", j=4, h=2
                )
                nc.vector.tensor_tensor(otv, data, rn, op=A.mult)
                if a == UPT - 1:
                    nc.sync.dma_start(orr[i], ot[:])

            SKEW = min(16, NH)
            sigTs = {}
            for h in range(min(SKEW, NH)):
                sigTs[h] = front_q(h)
            arsb = emit_allreduce()
            update_math()
            update_cast()
            for h in range(SKEW, NH):
                sigTs[h] = front_q(h)
                mid_q(h - SKEW)
            for h in range(max(0, NH - SKEW), NH):
                mid_q(h)

    nc.compile()
    return nc


_CACHE = {}


def _get_kernel(n_cores, tokens_per_core):
    key = (n_cores, tokens_per_core)
    if key not in _CACHE:
        _CACHE[key] = _build(n_cores, tokens_per_core)
    return _CACHE[key]


def _np_reference(queries, keys, values, M, z):
    """Fallback (is_empty edge case) — straight numpy port of the reference."""

    def elu1(x):
        return np.where(x > 0, x + 1.0, np.exp(np.minimum(x, 0.0)))

    def retrieve(sig, M, z):
        return (sig @ M) / ((sig @ z)[..., None] + EPS)

    sk = elu1(keys)
    existing = retrieve(sk, M, z)
    uv = values if z.sum() == 0 else values - existing
    dm = np.clip(
        np.einsum("bsd,bse->de", sk, uv) / (B * S), -MAX_DELTA, MAX_DELTA
    )
    dz = sk.sum(axis=(0, 1)) / B
    Mn = np.clip(M + dm, -MAX_MEMORY, MAX_MEMORY)
    zn = np.clip(z + dz, EPS, MAX_NORM)
    return retrieve(elu1(queries), Mn, zn).astype(np.float32)


def kernel(queries, keys, values, M, z, _want_results_obj=False, **_ignored):
    from concourse import bass_utils

    queries = np.ascontiguousarray(queries, dtype=np.float32)
    keys = np.ascontiguousarray(keys, dtype=np.float32)
    values = np.ascontiguousarray(values, dtype=np.float32)
    M = np.ascontiguousarray(M, dtype=np.float32)
    z = np.ascontiguousarray(z, dtype=np.float32)

    if float(z.sum()) == 0.0:
        # is_empty branch of the reference: update_values = values. Rare
        # (z all-zero); handled on host rather than in the kernel.
        return _np_reference(queries, keys, values, M, z)

    b, s, d = keys.shape
    tot = b * s
    tpc = tot // N_CORES
    nc = _get_kernel(N_CORES, tpc)

    kf = keys.reshape(tot, d)
    vf = values.reshape(tot, d)
    qf = queries.reshape(tot, d)
    z2 = z.reshape(d, 1)

    in_maps = []
    for c in range(N_CORES):
        sl = slice(c * tpc, (c + 1) * tpc)
        in_maps.append(
            {
                "keys": np.ascontiguousarray(kf[sl]),
                "values": np.ascontiguousarray(vf[sl]),
                "queries": np.ascontiguousarray(qf[sl]),
                "m": M,
                "z": z2,
            }
        )

    res = bass_utils.run_bass_kernel_spmd(
        nc, in_maps, core_ids=list(range(N_CORES))
    )
    out = np.concatenate(
        [np.asarray(res.results[c]["out"]).astype(np.float32)
         for c in range(N_CORES)],
        axis=0,
    ).reshape(b, s, d)
    if _want_results_obj:
        return out, res
    return out


# revision 20
# speedup vs baseline: 1.1472x; 1.1472x over previous
"""Trainium2 Bass kernel for BaseTensorMemory (delta-rule tensor memory).

Computes, for full inputs queries/keys/values [B,S,D], M [D,D], z [D]:
  sigma_k = elu(keys)+1 ; existing = (sigma_k@M)/(sigma_k@z+eps)
  delta_m = clip(einsum('bsd,bse->de', sigma_k, values-existing)/(B*S), +-1)
  delta_z = sigma_k.sum((0,1))/B
  M' = clip(M+delta_m, +-100); z' = clip(z+delta_z, eps, 1e6)
  out = (sigma_q@M')/(sigma_q@z'+eps)

Strategy: data-parallel over 8 NeuronCores. Flatten B*S tokens, shard
contiguously. Per core: keys pass accumulates [sigma^T@v | sigma^T@[1|ex]]
into PSUM, AllReduce the tiny [64,129] partial, build M'/z' on-chip, then
the queries retrieve pass streams the output.

Engine budget (measured): ACT = 1 elem/part/cycle @1.2GHz dtype-blind;
DVE = 1 elem/cycle @0.96GHz, 2x when every src/dst is 2-byte unit-stride
(tensor_scalar / tensor_tensor / copy only — scalar_tensor_tensor never).
So: q/k/v are DMA-cast f32->f16 on load (SWDGE), elu+1 is one ACT exp +
DVE ts(max,+1) + DVE tt(min), both 2x-eligible.  eps dropped (norms are
O(10)+); the ex sign is folded into a subtract at update time so the
reciprocal reads the PSUM norm column directly.

Device quirk found empirically: matmuls whose operands alternate base
partition (0 vs 64) inside one PSUM accumulation group hard-crash the
device (NRT_EXEC_UNIT_UNRECOVERABLE). Retrieval is therefore parity-banked:
even token-groups accumulate in bank 0 (operands at base partition 0), odd
groups in bank 1 (base partition 64).
"""

import numpy as np

B, S, D = 16, 16384, 64
N_CORES = 8
EPS = 1e-6
MAX_DELTA = 1.0
MAX_MEMORY = 100.0
MAX_NORM = 1e6

TILE_TOKENS = 4096  # macro-tile: [128, 2048] f16, four 1024-token units
QPM = TILE_TOKENS // 128  # 32 token-groups per macro-tile
UPT = 4  # units (psum-sized, 1024 tokens) per macro-tile
VW = 2 * D + 2  # 130: [v | 1 | ex | pad] block per group (pad: 4B align)


def _build(n_cores, tokens_per_core):
    import concourse.bacc as bacc
    import concourse.mybir as mybir
    import concourse.tile as tile
    from concourse import masks

    dt = mybir.dt
    f32, f16 = dt.float32, dt.float16
    A = mybir.AluOpType
    F = mybir.ActivationFunctionType

    T = tokens_per_core
    NT = T // TILE_TOKENS
    assert NT * TILE_TOKENS == T
    MFD = QPM * D  # 2048: macro-tile free dim

    nc = bacc.Bacc(
        "TRN2", target_bir_lowering=False, debug=False, num_devices=n_cores
    )
    k_d = nc.dram_tensor("keys", [T, D], f32, kind="ExternalInput").ap()
    v_d = nc.dram_tensor("values", [T, D], f32, kind="ExternalInput").ap()
    q_d = nc.dram_tensor("queries", [T, D], f32, kind="ExternalInput").ap()
    m_d = nc.dram_tensor("m", [D, D], f32, kind="ExternalInput").ap()
    z_d = nc.dram_tensor("z", [D, 1], f32, kind="ExternalInput").ap()
    o_d = nc.dram_tensor("out", [T, D], dt.bfloat16, kind="ExternalOutput").ap()

    kr = k_d.rearrange("(n p q) d -> n p (q d)", p=128, q=QPM)
    vr = v_d.rearrange("(n p q) d -> n p (q d)", p=128, q=QPM)
    qr = q_d.rearrange("(n p q) d -> n p (q d)", p=128, q=QPM)
    orr = o_d.rearrange("(n p q) d -> n p (q d)", p=128, q=QPM)

    with tile.TileContext(nc) as tc:
        with (
            tc.tile_pool(name="const", bufs=1) as cpool,
            tc.tile_pool(name="io", bufs=3) as io,
            tc.tile_pool(name="work", bufs=4) as work,
            tc.tile_pool(name="small", bufs=6) as small,
            tc.tile_pool(name="sigq", bufs=33) as sigq,
            tc.tile_pool(name="psT", bufs=3, space="PSUM") as psTp,
            tc.tile_pool(name="psR", bufs=2, space="PSUM") as psRp,
            tc.tile_pool(name="psA", bufs=1, space="PSUM") as psAp,
            tc.tile_pool(name="dram", bufs=1, space="DRAM") as dram,
        ):
            ident = cpool.tile([128, 128], f16)
            masks.make_identity(nc, ident[:])

            # [M|z] in fp16, replicated on both partition halves (parity-
            # banked retrieve uses operands at base partition 0 and 64).
            mz = cpool.tile([128, 65], f32)
            nc.sync.dma_start(mz[0:64, 0:64], m_d[:])
            nc.sync.dma_start(mz[0:64, 64:65], z_d[:])
            nc.sync.dma_start(mz[64:128, 0:64], m_d[:])
            nc.sync.dma_start(mz[64:128, 64:65], z_d[:])
            mz16 = cpool.tile([128, 65], f16)
            nc.scalar.copy(mz16[:], mz[:])

            AW = 2 * D + 1  # psA: [sv(64) | sum_sig | ex(64)]
            psA = psAp.tile([64, AW], f32)

            ncopy = [0]  # alternate sigT copies between DVE and ACT

            def elu_tile(xt):
                """sigma = min(exp(x), max(x,0)+1) -> [128, MFD] f16.
                One ACT pass + two 2x-eligible DVE passes (all-f16)."""
                e16 = work.tile([128, MFD], f16, tag="e")
                nc.scalar.activation(e16[:], xt[:], F.Exp)
                r1 = work.tile([128, MFD], f16, tag="r")
                nc.vector.tensor_scalar(
                    r1[:], xt[:], 0.0, 1.0, op0=A.max, op1=A.add
                )
                sig = work.tile([128, MFD], f16, tag="sig")
                nc.vector.tensor_tensor(sig[:], r1[:], e16[:], op=A.min)
                return sig

            def transpose_half(sig, a, copy_eng="alt", tag="sigT"):
                """4x [128,128] PE transposes of unit a -> sigT [128, 512]
                f16 (token-groups parity-stacked on partitions). start=True
                lazily zeroes the whole PSUM bank: first matmul only."""
                psT = psTp.tile([128, 512], f16, tag="psT")
                for c in range(4):
                    nc.tensor.matmul(
                        psT[:, c * 128 : (c + 1) * 128],
                        sig[:, a * 512 + c * 128 : a * 512 + (c + 1) * 128],
                        ident[:],
                        is_transpose=True,
                        start=(c == 0),
                        stop=(c == 3),
                    )
                pool = sigq if tag == "sigTq" else work
                sigT = pool.tile([128, 512], f16, tag=tag)
                if copy_eng == "alt":
                    eng = "act" if ncopy[0] % 2 else "dve"
                    ncopy[0] += 1
                else:
                    eng = copy_eng
                if eng == "act":
                    nc.scalar.copy(sigT[:], psT[:])
                else:
                    nc.vector.tensor_copy(sigT[:], psT[:])
                return sigT

            def retrieve_half(sigT, mztile):
                """Parity-banked: even groups -> bank0 (base 0), odd ->
                bank1 (base 64). In-unit group (h par, j) = 2j+h at bank h
                col 65j. Returns psum + data/norm views [128, h, j, *]."""
                psR = psRp.tile([128, 1024], f32, tag="psR")
                for g in range(8):
                    par, j = g % 2, g // 2
                    off = par * 512 + 65 * j
                    base = par * 64
                    lhsT = sigT[base : base + 64, j * 128 : (j + 1) * 128]
                    nc.tensor.matmul(
                        psR[:, off : off + 65],
                        lhsT,
                        mztile[base : base + 64, :],
                        start=(j == 0),
                        stop=(j == 3),
                    )
                ret = psR[:].rearrange("p (h x) -> p h x", h=2)[:, :, 0:260]
                ret = ret.rearrange("p h (j c) -> p h j c", j=4)
                return psR, ret[:, :, :, 0:64], ret[:, :, :, 64:65]

            def recip_norm(normv):
                """1/norm broadcast view [128, h, j, 64] f32 (eps dropped:
                norms are O(10)+ for keys, O(1e6) for queries)."""
                rn = small.tile([128, 8], f32, tag="rn")
                rnv = rn[:].rearrange("p (h j) -> p h j", h=2).unsqueeze(3)
                nc.vector.reciprocal(rnv, normv)
                return rnv.broadcast_to((128, 2, 4, 64))

            # ---------------- keys phase ----------------
            # Stage pipeline over 1024-token units:
            #   front(h): transposes + sigT copy
            #   mid(h):   retrieve + recip + ex-mul   (1 unit behind front)
            #   back(h):  16 delta matmuls            (2 units behind)
            NH = UPT * NT
            fronts = {}
            first_mm = [True]

            def front_k(h):
                i, a = h // UPT, h % UPT
                if a == 0:
                    kt = io.tile([128, MFD], f16, tag="kt")
                    nc.gpsimd.dma_start(kt[:], kr[i])  # f32->f16 cast DMA
                    vt = io.tile([128, MFD], f16, tag="vt")
                    nc.gpsimd.dma_start(vt[:], vr[i])  # f32->f16 cast DMA
                    sig = elu_tile(kt)
                    vox = io.tile([128, QPM * VW], f16, tag="vox")
                    voxg = vox[:].rearrange("p (g c) -> p g c", g=QPM)
                    # one delta matmul per group needs [v|1|ex] contiguous;
                    # values land via ACT copy (ACT has slack), ones memset.
                    nc.scalar.copy(voxg[:, :, 0:64], vt[:])
                    nc.gpsimd.memset(voxg[:, :, 64:65], 1.0)
                    fronts[i] = (sig, vox)
                sig, vox = fronts[i]
                sigT = transpose_half(sig, a)
                return sigT

            def mid_k(h, sigT):
                i, a = h // UPT, h % UPT
                sig, vox = fronts[i]
                psR, data, normv = retrieve_half(sigT, mz16)
                rn = recip_norm(normv)
                exv = vox[:, a * 8 * VW : (a + 1) * 8 * VW].rearrange(
                    "p (j h c) -> p h j c", j=4, h=2
                )[:, :, :, 65:129]
                nc.vector.tensor_tensor(exv, data, rn, op=A.mult)

            def back_k(h, last):
                i, a = h // UPT, h % UPT
                sig, vox = fronts[i]
                for g in range(8):
                    q = a * 8 + g
                    nc.tensor.matmul(
                        psA[:, 0 : 2 * D + 1],
                        sig[:, q * 64 : (q + 1) * 64],
                        vox[:, q * VW : q * VW + 2 * D + 1],
                        start=first_mm[0],
                        stop=(last and g == 7),
                    )
                    first_mm[0] = False

            # Deep skew: mid lags front by 2 units, back lags by 4, so each
            # cross-engine semaphore hop has slack and no engine stalls on
            # the just-produced tile.
            MSK, BSK = 1, 2
            stages = []
            for h in range(NH):
                stages.append(("f", h))
                if h >= MSK:
                    stages.append(("m", h - MSK))
                if h >= BSK:
                    stages.append(("b", h - BSK))
            for h in range(NH - MSK, NH):
                stages.append(("m", h))
            for h in range(NH - BSK, NH):
                stages.append(("b", h))
            sigTs = {}
            for kind, h in stages:
                if kind == "f":
                    sigTs[h] = front_k(h)
                elif kind == "m":
                    mid_k(h, sigTs[h])
                else:
                    back_k(h, last=(h == NH - 1))

            # ---------------- allreduce + update ----------------
            # Staging DMAs ride the sync (HWDGE) queue so the gpsimd queue
            # holds only collective_compute itself; the SKEW query fronts'
            # cast-DMAs are emitted BEFORE it (see below) so query loads
            # are not serialized behind the 16us collective.
            accsb = cpool.tile([64, AW], f32)
            nc.vector.tensor_copy(accsb[:], psA[:])
            arsb = accsb
            arin = arout = None
            if n_cores > 1:
                arin = dram.tile([64, AW], f32)
                arout = dram.tile([64, AW], f32)
                nc.sync.dma_start(arin[:], accsb[:])

            def emit_allreduce():
                if n_cores == 1:
                    return accsb
                nc.gpsimd.collective_compute(
                    "AllReduce",
                    mybir.AluOpType.add,
                    replica_groups=[list(range(n_cores))],
                    ins=[arin.opt()],
                    outs=[arout.opt()],
                )
                ar = cpool.tile([64, AW], f32)
                nc.sync.dma_start(ar[:], arout[:])
                return ar

            def update_math():
                # Fused DVE ops; emitted after SKEW queries fronts so the
                # DVE stream has front work while the AllReduce runs.
                # arsb = [sv(64) | sum_sig(1) | ex_sum(64)]
                # delta_m = clip((sv-ex)/(B*S), +-1); M' = clip(M+dm, +-100)
                nc.vector.tensor_tensor(
                    mzn[:, 0:64], arsb[:, 0:64], arsb[:, 65:129], op=A.subtract
                )
                nc.vector.tensor_scalar(
                    mzn[:, 0:64], mzn[:, 0:64], 1.0 / (B * S), MAX_DELTA,
                    op0=A.mult, op1=A.min,
                )
                nc.vector.scalar_tensor_tensor(
                    mzn[:, 0:64], mzn[:, 0:64], -MAX_DELTA, mz[0:64, 0:64],
                    op0=A.max, op1=A.add,
                )
                nc.vector.tensor_scalar(
                    mzn[:, 0:64], mzn[:, 0:64], MAX_MEMORY, -MAX_MEMORY,
                    op0=A.min, op1=A.max,
                )
                # delta_z = acc_z/B; z' = clip(z+dz, eps, 1e6)
                nc.vector.scalar_tensor_tensor(
                    mzn[:, 64:65], arsb[:, 64:65], 1.0 / B, mz[0:64, 64:65],
                    op0=A.mult, op1=A.add,
                )
                nc.vector.tensor_scalar(
                    mzn[:, 64:65], mzn[:, 64:65], EPS, MAX_NORM,
                    op0=A.max, op1=A.min,
                )

            mzn = cpool.tile([64, 65], f32)
            mzn128 = cpool.tile([128, 65], f32)
            mzn16 = cpool.tile([128, 65], f16)

            def update_cast():
                nc.gpsimd.dma_start(mzn128[0:64, :], mzn[:])
                nc.gpsimd.dma_start(mzn128[64:128, :], mzn[:])
                nc.gpsimd.tensor_copy(mzn16[:], mzn128[:])

            # ---------------- queries phase ----------------
            outs = {}
            nq = [0]

            def front_q(h):
                i, a = h // UPT, h % UPT
                if a == 0:
                    qt = io.tile([128, MFD], f16, tag="kt")
                    nc.gpsimd.dma_start(qt[:], qr[i])  # f32->f16 cast DMA
                    sig = elu_tile(qt)
                    fronts[i + NT] = (sig, None, None)
                sig, _, _ = fronts[i + NT]
                nq[0] += 1
                return transpose_half(sig, a, tag="sigTq")

            def mid_q(h):
                i, a = h // UPT, h % UPT
                psR, data, normv = retrieve_half(sigTs[h], mzn16)
                rn = recip_norm(normv)
                if a == 0:
                    ot = io.tile([128, MFD], dt.bfloat16, tag="ot")
                    outs[i] = ot
                ot = outs[i]
                otv = ot[:, a * 512 : (a + 1) * 512].rearrange(
                    "p (j h c) -> p h j c", j=4, h=2
                )
                nc.vector.tensor_tensor(otv, data, rn, op=A.mult)
                nc.sync.dma_start(
                    orr[i][:, a * 512 : (a + 1) * 512],
                    ot[:, a * 512 : (a + 1) * 512],
                )

            SKEW = min(32, NH)
            sigTs = {}
            for h in range(min(SKEW, NH)):
                sigTs[h] = front_q(h)
            arsb = emit_allreduce()
            update_math()
            update_cast()
            for h in range(SKEW, NH):
                sigTs[h] = front_q(h)
                mid_q(h - SKEW)
            for h in range(max(0, NH - SKEW), NH):
                mid_q(h)

    nc.compile()
    return nc


_CACHE = {}


def _get_kernel(n_cores, tokens_per_core):
    key = (n_cores, tokens_per_core)
    if key not in _CACHE:
        _CACHE[key] = _build(n_cores, tokens_per_core)
    return _CACHE[key]


def _np_reference(queries, keys, values, M, z):
    """Fallback (is_empty edge case) — straight numpy port of the reference."""

    def elu1(x):
        return np.where(x > 0, x + 1.0, np.exp(np.minimum(x, 0.0)))

    def retrieve(sig, M, z):
        return (sig @ M) / ((sig @ z)[..., None] + EPS)

    sk = elu1(keys)
    existing = retrieve(sk, M, z)
    uv = values if z.sum() == 0 else values - existing
    dm = np.clip(
        np.einsum("bsd,bse->de", sk, uv) / (B * S), -MAX_DELTA, MAX_DELTA
    )
    dz = sk.sum(axis=(0, 1)) / B
    Mn = np.clip(M + dm, -MAX_MEMORY, MAX_MEMORY)
    zn = np.clip(z + dz, EPS, MAX_NORM)
    return retrieve(elu1(queries), Mn, zn).astype(np.float32)


def kernel(queries, keys, values, M, z, _want_results_obj=False, **_ignored):
    from concourse import bass_utils

    queries = np.ascontiguousarray(queries, dtype=np.float32)
    keys = np.ascontiguousarray(keys, dtype=np.float32)
    values = np.ascontiguousarray(values, dtype=np.float32)
    M = np.ascontiguousarray(M, dtype=np.float32)
    z = np.ascontiguousarray(z, dtype=np.float32)

    if float(z.sum()) == 0.0:
        # is_empty branch of the reference: update_values = values. Rare
        # (z all-zero); handled on host rather than in the kernel.
        return _np_reference(queries, keys, values, M, z)

    b, s, d = keys.shape
    tot = b * s
    tpc = tot // N_CORES
    nc = _get_kernel(N_CORES, tpc)

    kf = keys.reshape(tot, d)
    vf = values.reshape(tot, d)
    qf = queries.reshape(tot, d)
    z2 = z.reshape(d, 1)

    in_maps = []
    for c in range(N_CORES):
        sl = slice(c * tpc, (c + 1) * tpc)
        in_maps.append(
            {
                "keys": np.ascontiguousarray(kf[sl]),
                "values": np.ascontiguousarray(vf[sl]),
                "queries": np.ascontiguousarray(qf[sl]),
                "m": M,
                "z": z2,
            }
        )

    res = bass_utils.run_bass_kernel_spmd(
        nc, in_maps, core_ids=list(range(N_CORES))
    )
    out = np.concatenate(
        [np.asarray(res.results[c]["out"]).astype(np.float32)
         for c in range(N_CORES)],
        axis=0,
    ).reshape(b, s, d)
    if _want_results_obj:
        return out, res
    return out
